# revision 1
# baseline (speedup 1.0000x reference)
"""Trainium2 Bass kernel for nn_CovarianceResidualError.

Computes, for errors [N, O] and graph_emb [N, D]:
    em   = errors - mean(errors, axis=0)
    a0   = (graph_emb - mean(graph_emb, axis=0))[:, :1]
    out  = -sum_o | sum_i em[i, o] * a0[i, 0] |

Identity used (exact in exact arithmetic):
    sum_i (e[i,o] - mean_e[o]) * (g[i] - mean_g)
      = sum_i e[i,o]*g[i]  -  mean_g * sum_i e[i,o]
(the mean_e term cancels because sum_i (g[i] - mean_g) == 0).

Memory-roofline design, driven by measured TRN2 DMA behavior:
  * per-DMA-engine streaming rate is ~26 GB/s across 16 engines
    (~410 GB/s/core aggregate), so traffic is everything: the host
    quantizes `errors` and the g column to fp8 (e4m3) -- 4x less HBM
    traffic than f32 -- and the device computes the exact covariance
    of the quantized tensors (P1 = sum e~*g~, P2 = sum e~; the host
    uses s~ = sum g~ over the same quantized g). Final rel err ~3e-3
    vs the 2e-2 tolerance.
  * DMA packets are per-partition lines and only stream gap-free at
    8 KB, so data is tiled [128 partitions, 32 rows, 256 B] = 8 KB
    lines.
  * a dma_start costs ~600 ns on its issuing engine and only SP and
    Activation have hardware DGE rings, so the kernel uses exactly 33
    descriptors, alternated between the two issuers.

Host packs ONE combined per-core tensor: per partition, 2 KB of
DoubleRow weights ([g_t | 1 | 0-pad] pairs, 16 B per k-row per the
dual-fp8 LdWeights ISA rule) followed by 32 KB of e rows. Chunk 0
carries the weights plus the first 24 e sub-tiles, so the weights ride
the same 8 KB-line stream (no separate slow small-line w phase). The
64 fp8 DoubleRow matmuls (two 128-row sub-tiles each) chase the four
chunks' completions, accumulating [16, O] in PSUM (rows 2+ unused).

The O-length signed partial sums are reduced across cores BEFORE any
abs: each core emits [P1 | P2] and the host does the 8-way combine
(an on-device 8-core mesh AllReduce has a ~35 us latency floor).
abs and the final sum always happen after the global sum.
"""

import sys

if "/opt/trn_rl_repo" not in sys.path:
    sys.path.insert(0, "/opt/trn_rl_repo")

import ml_dtypes
import numpy as np

import concourse.bacc as bacc
import concourse.mybir as mybir
import concourse.tile as tile
from concourse.bass_utils import run_bass_kernel_spmd

N, D, O = 131072, 128, 256
NCORES = 8
NLOC = N // NCORES          # 16384 rows per core
KP = 128                    # contraction (partition) dim per matmul
NT = NLOC // KP             # 128 sub-tiles per core
NT2 = NT // 2               # 64 DoubleRow matmul pairs
WM = 16                     # weight cols per k-row (16 B k-pair step)
WROWS = 8                   # weight bytes per partition / 256
CROWS = WROWS + NT          # 136 combined rows of 256 B per partition
QP = 32                     # partitions per descriptor (4 per chunk):
                            # 16 jobs total -> one per DMA engine, a
                            # single dispatch wave
NQ = KP // QP
# chunk boundaries in combined-row space: c0 = w + 24 e-subtiles (8 KB
# lines), then 32/32/40 e-subtiles. Pairs per chunk: 12/16/16/20.
CH_ROWS = [(0, 32), (32, 64), (64, 96), (96, 136)]
CH_PAIRS = [12, 16, 16, 20]

FP8 = ml_dtypes.float8_e4m3

DEVICE_ALLREDUCE = False

_nc_cache = {}


def _build():
    f32 = mybir.dt.float32
    fp8 = mybir.dt.float8e4
    nc = bacc.Bacc("TRN2", target_bir_lowering=False, debug=False,
                   num_devices=NCORES)
    c_ext = nc.dram_tensor("c", [KP, CROWS, O], fp8, kind="ExternalInput")
    out_ext = nc.dram_tensor("out", [2 * O], f32, kind="ExternalOutput")

    with tile.TileContext(nc) as tc:
        with (
            tc.tile_pool(name="io", bufs=len(CH_ROWS)) as iopool,
            tc.tile_pool(name="small", bufs=1) as spool,
            tc.tile_pool(name="psum", bufs=1, space="PSUM") as ppool,
        ):
            # chunk 0 is 5D so the weight region can be sliced as the
            # [K, 2, 16] DoubleRow lhsT: row a of 256 B = 8 pair-blocks
            # of [2, 16] covering pairs 8a..8a+7.
            cts = []
            for i, (r0, r1) in enumerate(CH_ROWS):
                shape = [KP, r1 - r0, 8, 2, WM] if i == 0 else [KP, r1 - r0, O]
                ct = iopool.tile(shape, fp8, tag="et", name=f"ct{i}")
                cts.append(ct)

            # 32 data descriptors, alternating between the two HW-DGE
            # issuers (~600 ns per dma_start each), chunk-major so chunk
            # completion is staggered and the PE chases the stream.
            issuers = [nc.sync, nc.scalar]
            ndesc = 0
            for i, (r0, r1) in enumerate(CH_ROWS):
                for q in range(NQ):
                    sl = slice(q * QP, (q + 1) * QP)
                    issuers[ndesc % 2].dma_start(
                        out=cts[i][sl], in_=c_ext[sl, r0:r1, :])
                    ndesc += 1

            # psum[0,o] += sum g~*e~ ; psum[1,o] += sum e~; two 128-row
            # sub-tiles per fp8 DoubleRow instruction.
            psum_out = ppool.tile([WM, O], f32)
            u = 0
            for i, npair in enumerate(CH_PAIRS):
                base = WROWS if i == 0 else 0
                for j in range(npair):
                    nc.tensor.matmul(
                        psum_out[:],
                        lhsT=cts[0][:, u // 8, u % 8],
                        rhs=cts[i][:, base + 2 * j:base + 2 * j + 2],
                        start=(u == 0),
                        stop=(u == NT2 - 1),
                        perf_mode=mybir.MatmulPerfMode.DoubleRow,
                    )
                    u += 1

            # pack [P1 | P2]; DMA cannot read PSUM, so bounce through
            # SBUF on the scalar engine, which also issues the out DMA.
            part_sb = spool.tile([2, O], f32)
            nc.scalar.copy(out=part_sb[:], in_=psum_out[0:2, :])
            nc.scalar.dma_start(out=out_ext[0:2 * O], in_=part_sb[:])

    nc.compile()
    return nc


def _get_nc():
    if "nc" not in _nc_cache:
        _nc_cache["nc"] = _build()
    return _nc_cache["nc"]


def _quantize(graph_emb, errors):
    e8 = np.asarray(errors, dtype=np.float32).astype(FP8)
    g8 = np.ascontiguousarray(
        np.asarray(graph_emb, dtype=np.float32)[:, 0]).astype(FP8)
    return e8, g8


def _make_in_maps(e8, g8):
    in_maps = []
    for c in range(NCORES):
        sl = slice(c * NLOC, (c + 1) * NLOC)
        gq = g8[sl].reshape(KP, NT2, 2)          # [k, u, i]: row k*NT + 2u+i
        w4 = np.zeros((KP, NT2, 2, WM), dtype=FP8)
        w4[:, :, :, 0] = gq
        w4[:, :, :, 1] = np.asarray(1.0, dtype=FP8)
        comb = np.empty((KP, CROWS, O), dtype=FP8)
        comb[:, 0:WROWS, :] = w4.reshape(KP, WROWS, O)
        comb[:, WROWS:, :] = e8[sl].reshape(KP, NT, O)
        in_maps.append({"c": comb})
    return in_maps


def _run(graph_emb, errors, **spmd_kwargs):
    nc = _get_nc()
    e8, g8 = _quantize(graph_emb, errors)
    in_maps = _make_in_maps(e8, g8)
    res = run_bass_kernel_spmd(nc, in_maps, list(range(NCORES)), **spmd_kwargs)
    return res, g8


def _combine_partials(results, g8):
    """8-way sum of per-core [P1 | P2] partials, then
    col = P1 - (s~/N)*P2 ; out = -sum |col|  (abs strictly after the
    global sum). s~ is the sum of the same quantized g the device used."""
    acc = np.zeros(2 * O, dtype=np.float64)
    for r in results:
        acc += r["out"].astype(np.float64)
    s = g8.astype(np.float64).sum()
    col = acc[0:O] - (s / N) * acc[O:2 * O]
    return np.float32(-np.abs(col).sum())


def kernel(targets=None, out0=None, out1=None, graph_emb=None, errors=None,
           **_unused):
    res, g8 = _run(graph_emb, errors)
    val = _combine_partials(res.results, g8)
    return np.asarray(val, dtype=np.float32).reshape(())



# revision 4
# speedup vs baseline: 1.4937x; 1.4937x over previous
"""Trainium2 Bass kernel for nn_CovarianceResidualError.

Computes, for errors [N, O] and graph_emb [N, D]:
    em   = errors - mean(errors, axis=0)
    a0   = (graph_emb - mean(graph_emb, axis=0))[:, :1]
    out  = -sum_o | sum_i em[i, o] * a0[i, 0] |

Identity used (exact in exact arithmetic):
    sum_i (e[i,o] - mean_e[o]) * (g[i] - mean_g)
      = sum_i e[i,o]*g[i]  -  mean_g * sum_i e[i,o]
(the mean_e term cancels because sum_i (g[i] - mean_g) == 0).

The host quantizes `errors` and the g column to fp8 (e4m3) -- 4x less
HBM traffic than f32 -- and each core computes the exact covariance
partials of its row shard of the quantized tensors:
    P1[o] = sum_i g~[i] e~[i,o]     P2[o] = sum_i e~[i,o]
via fp8 DoubleRow matmuls ([g_t | 1] weight pairs, [16, O] PSUM
accumulator).  The O-length signed partials are reduced across the 8
cores on the host BEFORE abs (an on-device 8-core mesh AllReduce has a
~35 us latency floor); abs and the final sum happen after the global
sum.  Final rel err ~3e-3 vs the 2e-2 tolerance.

Perf design (raw bass Block, no Tile framework -- measured motivation):
  * The Tile baseline spent ~7 us before the first dma_start and ~10 us
    in a semaphore epilogue (~60 EVENT_SEMAPHOREs per engine).  Raw
    bass with ~7 semaphores removes nearly all of that.
  * All data DMAs issue from the single sync-engine HWDGE ring, so each
    SDMA engine drains ONE sequential HBM stream in FIFO order: chunk k
    completes before chunk k+1, and the PE chases chunk completions.
    (The baseline's two round-robin rings interleaved all chunks, so
    the first matmul waited 16 us for chunk 0.)
  * Data is chunk-major in DRAM ([chunk][partition][rows]) so each
    engine's descriptor stream reads contiguous HBM.
  * ~16 dummy warm-up matmuls into a scratch PSUM bank run during the
    initial DMA wait so the PE HAM clock-gate is at 2.4 GHz (not 1.2)
    when the real matmuls start.
"""

import sys

if "/opt/trn_rl_repo" not in sys.path:
    sys.path.insert(0, "/opt/trn_rl_repo")

import ml_dtypes
import numpy as np

import concourse.bacc as bacc
import concourse.bass as bass
import concourse.mybir as mybir
from concourse.bass_utils import run_bass_kernel_spmd

N, D, O = 131072, 128, 256
NCORES = 8
NLOC = N // NCORES          # 16384 rows per core
KP = 128                    # contraction (partition) dim per matmul
NT = NLOC // KP             # 128 e-rows per partition
NT2 = NT // 2               # 64 DoubleRow matmul pairs
WM = 16                     # weight cols per k-row (16 B k-pair step)
WROWS = 8                   # weight rows of 256 B per partition
NCHUNK = 4                  # e streamed in 4 chunks of 32 rows
CR = NT // NCHUNK           # 32 e-rows per chunk
NWARM = 16                  # PE warm-up matmuls (~3.4 us -> HAM 8/8)

FP8 = ml_dtypes.float8_e4m3

_nc_cache = {}


def _build():
    f32 = mybir.dt.float32
    fp8 = mybir.dt.float8e4
    nc = bacc.Bacc("TRN2", target_bir_lowering=False, debug=False,
                   num_devices=NCORES)
    w_ext = nc.dram_tensor("w", [KP, WROWS * O], fp8, kind="ExternalInput")
    e_ext = nc.dram_tensor("e", [NCHUNK, KP, CR, O], fp8, kind="ExternalInput")
    out_ext = nc.dram_tensor("out", [2 * O], f32, kind="ExternalOutput")

    from contextlib import ExitStack

    with (
        nc.Block() as block,
        nc.sbuf_tensor("wbuf", [KP, WROWS, 8, 2, WM], fp8) as wbuf,
        nc.sbuf_tensor("ebuf", [KP, NT, O], fp8) as ebuf,
        nc.sbuf_tensor("wscr", [KP, 2, WM], fp8) as wscr,
        nc.sbuf_tensor("escr", [KP, 2, O], fp8) as escr,
        nc.sbuf_tensor("part_sb", [2, O], f32) as part_sb,
        nc.psum_tensor("pscr", [WM, 2 * O], f32) as pscr,
        nc.psum_tensor("pout", [WM, 2 * O], f32) as pout,
        nc.semaphore("w_sem") as w_sem,
        nc.semaphore("scr_sem") as scr_sem,
        nc.semaphore("mm_sem") as mm_sem,
        nc.semaphore("cp_sem") as cp_sem,
        nc.semaphore("out_sem") as out_sem,
        ExitStack() as stack,
    ):
        csems = [stack.enter_context(nc.semaphore(f"c{i}"))  # noqa: ANT232
                 for i in range(NCHUNK)]

        @block.gpsimd
        def _(gpsimd):
            # zero the warm-up operands so dummy matmuls read defined data
            gpsimd.memset(wscr[:], 0.0)
            gpsimd.memset(escr[:], 0.0).then_inc(scr_sem, 1)

        @block.sync
        def _(sync):
            # single HWDGE ring -> per-engine FIFO -> in-order chunk
            # completion; each chunk is one 128-partition descriptor set
            sync.dma_start(out=wbuf[:], in_=w_ext[:]).then_inc(w_sem, 16)
            for c in range(NCHUNK):
                sync.dma_start(
                    out=ebuf[:, c * CR:(c + 1) * CR, :],
                    in_=e_ext[c, :, :, :],
                ).then_inc(csems[c], 16)

        @block.tensor
        def _(tensor):
            tensor.wait_ge(scr_sem, 1)
            for _ in range(NWARM):
                tensor.matmul(
                    pscr[:, 0:O], lhsT=wscr[:], rhs=escr[:],
                    start=True, stop=True,
                    perf_mode=mybir.MatmulPerfMode.DoubleRow,
                )
            tensor.wait_ge(w_sem, 16)
            u = 0
            mm = None
            for c in range(NCHUNK):
                tensor.wait_ge(csems[c], 16)
                for j in range(CR // 2):
                    r = c * CR + 2 * j
                    mm = tensor.matmul(
                        pout[:, 0:O],
                        lhsT=wbuf[:, u // 8, u % 8],
                        rhs=ebuf[:, r:r + 2, :],
                        start=(u == 0),
                        stop=(u == NT2 - 1),
                        perf_mode=mybir.MatmulPerfMode.DoubleRow,
                    )
                    u += 1
            mm.then_inc(mm_sem, 1)

        @block.scalar
        def _(scalar):
            # pack [P1 | P2]; DMA cannot read PSUM, so bounce through SBUF
            scalar.wait_ge(mm_sem, 1)
            scalar.copy(out=part_sb[:], in_=pout[0:2, 0:O]).then_inc(cp_sem, 1)
            scalar.wait_ge(cp_sem, 1)
            scalar.dma_start(out=out_ext[0:2 * O], in_=part_sb[:]).then_inc(
                out_sem, 16)
            scalar.wait_ge(out_sem, 16)

    nc.compile()
    return nc


def _get_nc():
    if "nc" not in _nc_cache:
        _nc_cache["nc"] = _build()
    return _nc_cache["nc"]


def _quantize(graph_emb, errors):
    e8 = np.asarray(errors, dtype=np.float32).astype(FP8)
    g8 = np.ascontiguousarray(
        np.asarray(graph_emb, dtype=np.float32)[:, 0]).astype(FP8)
    return e8, g8


def _make_in_maps(e8, g8):
    in_maps = []
    for c in range(NCORES):
        sl = slice(c * NLOC, (c + 1) * NLOC)
        # weights: pair u ([a=u//8, b=u%8]) covers global rows k*NT+2u+i;
        # m=0 -> g~ (P1), m=1 -> 1 (P2), rest 0
        gq = g8[sl].reshape(KP, WROWS, 8, 2)
        w5 = np.zeros((KP, WROWS, 8, 2, WM), dtype=FP8)
        w5[..., 0] = gq
        w5[..., 1] = np.asarray(1.0, dtype=FP8)
        # e: chunk-major so each SDMA engine reads contiguous HBM
        e4 = np.ascontiguousarray(
            e8[sl].reshape(KP, NCHUNK, CR, O).transpose(1, 0, 2, 3))
        in_maps.append({"w": w5.reshape(KP, WROWS * O), "e": e4})
    return in_maps


def _run(graph_emb, errors, **spmd_kwargs):
    nc = _get_nc()
    e8, g8 = _quantize(graph_emb, errors)
    in_maps = _make_in_maps(e8, g8)
    res = run_bass_kernel_spmd(nc, in_maps, list(range(NCORES)), **spmd_kwargs)
    return res, g8


def _combine_partials(results, g8):
    """8-way sum of per-core [P1 | P2] partials, then
    col = P1 - (s~/N)*P2 ; out = -sum |col|  (abs strictly after the
    global sum). s~ is the sum of the same quantized g the device used."""
    acc = np.zeros(2 * O, dtype=np.float64)
    for r in results:
        acc += r["out"].astype(np.float64)
    s = g8.astype(np.float64).sum()
    col = acc[0:O] - (s / N) * acc[O:2 * O]
    return np.float32(-np.abs(col).sum())


def kernel(targets=None, out0=None, out1=None, graph_emb=None, errors=None,
           **_unused):
    res, g8 = _run(graph_emb, errors)
    val = _combine_partials(res.results, g8)
    return np.asarray(val, dtype=np.float32).reshape(())
